# revision 1
# baseline (speedup 1.0000x reference)
"""Experts-choose-contract MoE kernel for Trainium2 (8 NeuronCores).

Problem: x (B=4, T=4096, D=1024) f32; expert_indices (B, E=8, C=1024);
weight (E, O=512, D); bias (E, O).
out[b, e, c, :] = x[b, expert_indices[b, e, c], :] @ weight[e].T + bias[e]

Sharding: expert-parallel — core e handles expert e. x is replicated; each
core gathers its expert's (B*C = 4096) token rows from HBM with dma_gather,
transposes token tiles on the PE (contract dim must sit on partitions),
runs the grouped GEMM (4096x1024 @ 1024x512) accumulating in PSUM, adds
bias, and writes its (4096, 512) slice. The host stacks the 8 slices.

Modes (env BASSK_MODE): "f32" exact fp32 matmul (4 cyc/row),
"f32r" (default) single-pass fp32 matmul (1 cyc/row at N>=512),
"bf16" host-casts x/w to bf16 and gathers pre-transposed (2-byte dtype
supports transposing gather), skipping the PE transposes entirely.
"""

import os

import numpy as np

import concourse.bass as bass
import concourse.mybir as mybir
import concourse.tile as tile
from concourse import bacc
from concourse.bass_utils import run_bass_kernel_spmd
from concourse.masks import make_identity

B, T, D = 4, 4096, 1024
E, C, O = 8, 1024, 512
BT = B * T          # 16384 rows in flattened x
NTOK = B * C        # 4096 tokens gathered per expert/core
KT = D // 128       # 8 contraction tiles
CHUNK = 512         # tokens per dma_gather
NCHUNK = NTOK // CHUNK
IDX_COLS = NTOK // 16

MODE = os.environ.get("BASSK_MODE", "f32r")


def build_nc(mode=MODE):
    nc = bacc.Bacc("TRN2", target_bir_lowering=False, debug=False)
    f32 = mybir.dt.float32
    bf16 = mybir.dt.bfloat16
    i16 = mybir.dt.int16

    if mode == "bf16":
        x_dram = nc.dram_tensor("x", [BT, D], bf16, kind="ExternalInput")
        wt_dram = nc.dram_tensor("wt", [128, KT, O], bf16, kind="ExternalInput")
    else:
        x_dram = nc.dram_tensor("x", [BT, D], f32, kind="ExternalInput")
        wt_dram = nc.dram_tensor("wt", [128, KT, O], f32, kind="ExternalInput")
    idx_dram = nc.dram_tensor("idx", [128, IDX_COLS], i16, kind="ExternalInput")
    bias_dram = nc.dram_tensor("bias", [O], f32, kind="ExternalInput")
    out_dram = nc.dram_tensor("out", [NTOK, O], f32, kind="ExternalOutput")

    mm_dt = {
        "f32": f32,
        "f32r": mybir.dt.float32r,
        "bf16": bf16,
    }[mode]

    with tile.TileContext(nc) as tc:
        with (
            tc.tile_pool(name="singles", bufs=1) as singles,
            tc.tile_pool(name="gpool", bufs=2) as gpool,
            tc.tile_pool(name="tpool", bufs=3) as tpool,
            tc.tile_pool(name="opool", bufs=3) as opool,
            tc.tile_pool(name="psum_t", bufs=4, space="PSUM") as psum_t,
            tc.tile_pool(name="psum_mm", bufs=3, space="PSUM") as psum_mm,
        ):
            wt_sb = singles.tile([128, KT, O], wt_dram.dtype)
            nc.sync.dma_start(wt_sb, wt_dram.ap())
            bias_sb = singles.tile([128, O], f32)
            nc.sync.dma_start(
                bias_sb,
                bass.AP(tensor=bias_dram, offset=0, ap=[[0, 128], [1, O]]),
            )
            idx_sb = singles.tile([128, IDX_COLS], i16)
            nc.sync.dma_start(idx_sb, idx_dram.ap())
            if mode != "bf16":
                ident = singles.tile([128, 128], f32)
                make_identity(nc, ident)

            for c in range(NCHUNK):
                icols = CHUNK // 16
                idx_slice = idx_sb[:, c * icols : (c + 1) * icols]
                if mode == "bf16":
                    # transposing gather: g[p, k, t] = x[tok_t, k*128 + p]
                    g = gpool.tile([128, KT, CHUNK], bf16)
                    nc.gpsimd.dma_gather(
                        out_ap=g[:],
                        in_ap=x_dram.ap(),
                        idxs_ap=idx_slice,
                        num_idxs=CHUNK,
                        num_idxs_reg=CHUNK,
                        elem_size=D,
                        transpose=True,
                    )
                else:
                    # g[p, j, :] = token row (c*CHUNK + j*128 + p)
                    g = gpool.tile([128, CHUNK // 128, D], f32)
                    nc.gpsimd.dma_gather(
                        out_ap=g[:],
                        in_ap=x_dram.ap(),
                        idxs_ap=idx_slice,
                        num_idxs=CHUNK,
                        num_idxs_reg=CHUNK,
                        elem_size=D,
                    )

                if mode == "bf16":
                    # matmul directly from the transposed gather, 512-token N
                    # split into PSUM-bank-sized 512 outputs: out tile is
                    # [tok, O] so tokens must be the PSUM partition dim ->
                    # need lhsT = tokens. g[:, k, :] is [d128, tok512];
                    # use it as lhsT in 128-token column slices.
                    for j in range(CHUNK // 128):
                        pso = psum_mm.tile([128, O], f32)
                        for k in range(KT):
                            nc.tensor.matmul(
                                pso,
                                lhsT=g[:, k, j * 128 : (j + 1) * 128],
                                rhs=wt_sb[:, k, :],
                                start=(k == 0),
                                stop=(k == KT - 1),
                            )
                        ot = opool.tile([128, O], f32)
                        nc.vector.tensor_add(ot, pso, bias_sb)
                        t = c * (CHUNK // 128) + j
                        nc.sync.dma_start(
                            out_dram.ap()[t * 128 : (t + 1) * 128, :], ot
                        )
                else:
                    for j in range(CHUNK // 128):
                        tokT = tpool.tile([128, KT, 128], f32)
                        for k in range(KT):
                            pst = psum_t.tile([128, 128], f32)
                            nc.tensor.transpose(
                                pst, g[:, j, k * 128 : (k + 1) * 128], ident
                            )
                            # alternate copy engines to split the PSUM->SBUF load
                            if k % 2 == 0:
                                nc.vector.tensor_copy(tokT[:, k, :], pst)
                            else:
                                nc.scalar.copy(tokT[:, k, :], pst)
                        pso = psum_mm.tile([128, O], f32)
                        for k in range(KT):
                            nc.tensor.matmul(
                                pso,
                                lhsT=tokT[:, k, :].bitcast(mm_dt),
                                rhs=wt_sb[:, k, :].bitcast(mm_dt),
                                start=(k == 0),
                                stop=(k == KT - 1),
                            )
                        ot = opool.tile([128, O], f32)
                        nc.vector.tensor_add(ot, pso, bias_sb)
                        t = c * (CHUNK // 128) + j
                        nc.sync.dma_start(
                            out_dram.ap()[t * 128 : (t + 1) * 128, :], ot
                        )

    nc.compile()
    return nc


def prepare_in_maps(x, expert_indices, weight, bias, mode=MODE):
    x = np.ascontiguousarray(np.asarray(x, dtype=np.float32).reshape(BT, D))
    idx = np.asarray(expert_indices).astype(np.int64)
    w = np.asarray(weight, dtype=np.float32)
    b = np.asarray(bias, dtype=np.float32)

    if mode == "bf16":
        import ml_dtypes

        x_dev = x.astype(ml_dtypes.bfloat16)
    else:
        x_dev = x

    in_maps = []
    for e in range(E):
        flat = (idx[:, e, :] + (np.arange(B) * T)[:, None]).reshape(-1)
        assert flat.min() >= 0 and flat.max() < BT
        flat = flat.astype(np.int16)
        # per-chunk wrap: index i of chunk c sits at [i % 16, c*icols + i//16],
        # replicated to all 8 Q7 core groups (128 partitions).
        wrapped = flat.reshape(NCHUNK, CHUNK // 16, 16).transpose(2, 0, 1)
        idx_arr = np.tile(wrapped.reshape(16, IDX_COLS), (8, 1)).copy()
        wt = np.ascontiguousarray(
            w[e].T.reshape(KT, 128, O).transpose(1, 0, 2)
        )  # [128, KT, O]; wt[p, k, o] = w[e, o, k*128+p]
        if mode == "bf16":
            import ml_dtypes

            wt = wt.astype(ml_dtypes.bfloat16)
        in_maps.append(
            {"x": x_dev, "idx": idx_arr, "wt": wt, "bias": b[e].copy()}
        )
    return in_maps


_NC_CACHE = {}


def _get_nc(mode=MODE):
    if mode not in _NC_CACHE:
        _NC_CACHE[mode] = build_nc(mode)
    return _NC_CACHE[mode]


def kernel(x, expert_indices, weight, bias, _collect=None):
    nc = _get_nc()
    in_maps = prepare_in_maps(x, expert_indices, weight, bias)
    kwargs = {}
    if _collect is not None:
        kwargs = _collect.pop("kwargs", {})
    res = run_bass_kernel_spmd(nc, in_maps, core_ids=list(range(E)), **kwargs)
    if _collect is not None:
        _collect["res"] = res
    out = np.stack(
        [res.results[e]["out"].reshape(B, C, O) for e in range(E)], axis=1
    )
    return out


# revision 3
# speedup vs baseline: 31.6175x; 31.6175x over previous
"""Experts-choose-contract MoE kernel for Trainium2 (8 NeuronCores).

Problem: x (B=4, T=4096, D=1024) f32; expert_indices (B, E=8, C=1024);
weight (E, O=512, D); bias (E, O).
out[b, e, c, :] = x[b, expert_indices[b, e, c], :] @ weight[e].T + bias[e]

Sharding: expert-parallel — core e handles expert e. x is replicated; each
core gathers its expert's (B*C = 4096) token rows from HBM with dma_gather,
transposes token tiles on the PE (contract dim must sit on partitions),
runs the grouped GEMM (4096x1024 @ 1024x512) accumulating in PSUM, adds
bias, and writes its (4096, 512) slice. The host stacks the 8 slices.

Modes (env BASSK_MODE): "f32" exact fp32 matmul (4 cyc/row),
"f32r" (default) single-pass fp32 matmul (1 cyc/row at N>=512),
"bf16" host-casts x/w to bf16 and gathers pre-transposed (2-byte dtype
supports transposing gather), skipping the PE transposes entirely.
"""

import os

import numpy as np

import concourse.bass as bass
import concourse.mybir as mybir
import concourse.tile as tile
from concourse import bacc
from concourse.bass_utils import run_bass_kernel_spmd
from concourse.masks import make_identity

B, T, D = 4, 4096, 1024
E, C, O = 8, 1024, 512
BT = B * T          # 16384 rows in flattened x
NTOK = B * C        # 4096 tokens gathered per expert/core
KT = D // 128       # 8 contraction tiles
CHUNK = 512         # tokens per dma_gather
NCHUNK = NTOK // CHUNK
IDX_COLS = NTOK // 16

MODE = os.environ.get("BASSK_MODE", "f32r")


def build_nc(mode=MODE, repeat=1):
    nc = bacc.Bacc("TRN2", target_bir_lowering=False, debug=False)
    f32 = mybir.dt.float32
    bf16 = mybir.dt.bfloat16
    i16 = mybir.dt.int16

    if mode == "bf16":
        x_dram = nc.dram_tensor("x", [BT, D], bf16, kind="ExternalInput")
        wt_dram = nc.dram_tensor("wt", [128, KT, O], bf16, kind="ExternalInput")
    else:
        x_dram = nc.dram_tensor("x", [BT, D], f32, kind="ExternalInput")
        wt_dram = nc.dram_tensor("wt", [128, KT, O], f32, kind="ExternalInput")
    idx_dram = nc.dram_tensor("idx", [128, IDX_COLS], i16, kind="ExternalInput")
    bias_dram = nc.dram_tensor("bias", [O], f32, kind="ExternalInput")
    out_dram = nc.dram_tensor("out", [NTOK, O], f32, kind="ExternalOutput")

    mm_dt = {
        "f32": f32,
        "f32r": mybir.dt.float32r,
        "bf16": bf16,
    }[mode]

    with tile.TileContext(nc) as tc:
        with (
            tc.tile_pool(name="singles", bufs=1) as singles,
            tc.tile_pool(name="gpool", bufs=2) as gpool,
            tc.tile_pool(name="tpool", bufs=3) as tpool,
            tc.tile_pool(name="opool", bufs=3) as opool,
            tc.tile_pool(name="psum_t", bufs=4, space="PSUM") as psum_t,
            tc.tile_pool(name="psum_mm", bufs=3, space="PSUM") as psum_mm,
        ):
            wt_sb = singles.tile([128, KT, O], wt_dram.dtype)
            nc.sync.dma_start(wt_sb, wt_dram.ap())
            bias_sb = singles.tile([128, O], f32)
            nc.sync.dma_start(
                bias_sb,
                bass.AP(tensor=bias_dram, offset=0, ap=[[0, 128], [1, O]]),
            )
            idx_sb = singles.tile([128, IDX_COLS], i16)
            nc.sync.dma_start(idx_sb, idx_dram.ap())
            if mode != "bf16":
                ident = singles.tile([128, 128], f32)
                make_identity(nc, ident)

            for c in range(NCHUNK * repeat):
                c = c % NCHUNK
                icols = CHUNK // 16
                idx_slice = idx_sb[:, c * icols : (c + 1) * icols]
                if mode == "bf16":
                    # transposing gather: g[p, k, t] = x[tok_t, k*128 + p]
                    g = gpool.tile([128, KT, CHUNK], bf16)
                    nc.gpsimd.dma_gather(
                        out_ap=g[:],
                        in_ap=x_dram.ap(),
                        idxs_ap=idx_slice,
                        num_idxs=CHUNK,
                        num_idxs_reg=CHUNK,
                        elem_size=D,
                        transpose=True,
                    )
                else:
                    # g[p, j, :] = token row (c*CHUNK + j*128 + p)
                    g = gpool.tile([128, CHUNK // 128, D], f32)
                    nc.gpsimd.dma_gather(
                        out_ap=g[:],
                        in_ap=x_dram.ap(),
                        idxs_ap=idx_slice,
                        num_idxs=CHUNK,
                        num_idxs_reg=CHUNK,
                        elem_size=D,
                    )

                if mode == "bf16":
                    # matmul directly from the transposed gather, 512-token N
                    # split into PSUM-bank-sized 512 outputs: out tile is
                    # [tok, O] so tokens must be the PSUM partition dim ->
                    # need lhsT = tokens. g[:, k, :] is [d128, tok512];
                    # use it as lhsT in 128-token column slices.
                    for j in range(CHUNK // 128):
                        pso = psum_mm.tile([128, O], f32)
                        for k in range(KT):
                            nc.tensor.matmul(
                                pso,
                                lhsT=g[:, k, j * 128 : (j + 1) * 128],
                                rhs=wt_sb[:, k, :],
                                start=(k == 0),
                                stop=(k == KT - 1),
                            )
                        ot = opool.tile([128, O], f32)
                        nc.vector.tensor_add(ot, pso, bias_sb)
                        t = c * (CHUNK // 128) + j
                        nc.sync.dma_start(
                            out_dram.ap()[t * 128 : (t + 1) * 128, :], ot
                        )
                else:
                    for j in range(CHUNK // 128):
                        tokT = tpool.tile([128, KT, 128], f32)
                        for k in range(KT):
                            pst = psum_t.tile([128, 128], f32)
                            nc.tensor.transpose(
                                pst, g[:, j, k * 128 : (k + 1) * 128], ident
                            )
                            # alternate copy engines to split the PSUM->SBUF load
                            if k % 2 == 0:
                                nc.vector.tensor_copy(tokT[:, k, :], pst)
                            else:
                                nc.scalar.copy(tokT[:, k, :], pst)
                        pso = psum_mm.tile([128, O], f32)
                        for k in range(KT):
                            nc.tensor.matmul(
                                pso,
                                lhsT=tokT[:, k, :].bitcast(mm_dt),
                                rhs=wt_sb[:, k, :].bitcast(mm_dt),
                                start=(k == 0),
                                stop=(k == KT - 1),
                            )
                        ot = opool.tile([128, O], f32)
                        nc.vector.tensor_add(ot, pso, bias_sb)
                        t = c * (CHUNK // 128) + j
                        nc.sync.dma_start(
                            out_dram.ap()[t * 128 : (t + 1) * 128, :], ot
                        )

    nc.compile()
    return nc


def prepare_in_maps(x, expert_indices, weight, bias, mode=MODE):
    x = np.ascontiguousarray(np.asarray(x, dtype=np.float32).reshape(BT, D))
    idx = np.asarray(expert_indices).astype(np.int64)
    w = np.asarray(weight, dtype=np.float32)
    b = np.asarray(bias, dtype=np.float32)

    if mode == "bf16":
        import ml_dtypes

        x_dev = x.astype(ml_dtypes.bfloat16)
    else:
        x_dev = x

    in_maps = []
    for e in range(E):
        flat = (idx[:, e, :] + (np.arange(B) * T)[:, None]).reshape(-1)
        assert flat.min() >= 0 and flat.max() < BT
        flat = flat.astype(np.int16)
        # per-chunk wrap: index i of chunk c sits at [i % 16, c*icols + i//16],
        # replicated to all 8 Q7 core groups (128 partitions).
        wrapped = flat.reshape(NCHUNK, CHUNK // 16, 16).transpose(2, 0, 1)
        idx_arr = np.tile(wrapped.reshape(16, IDX_COLS), (8, 1)).copy()
        wt = np.ascontiguousarray(
            w[e].T.reshape(KT, 128, O).transpose(1, 0, 2)
        )  # [128, KT, O]; wt[p, k, o] = w[e, o, k*128+p]
        if mode == "bf16":
            import ml_dtypes

            wt = wt.astype(ml_dtypes.bfloat16)
        in_maps.append(
            {"x": x_dev, "idx": idx_arr, "wt": wt, "bias": b[e].copy()}
        )
    return in_maps


_NC_CACHE = {}


def _get_nc(mode=MODE):
    if mode not in _NC_CACHE:
        _NC_CACHE[mode] = build_nc(mode)
    return _NC_CACHE[mode]


def kernel(x, expert_indices, weight, bias, _collect=None):
    nc = _get_nc()
    in_maps = prepare_in_maps(x, expert_indices, weight, bias)
    kwargs = {}
    if _collect is not None:
        kwargs = _collect.pop("kwargs", {})
    res = run_bass_kernel_spmd(nc, in_maps, core_ids=list(range(E)), **kwargs)
    if _collect is not None:
        _collect["res"] = res
    out = np.stack(
        [res.results[e]["out"].reshape(B, C, O) for e in range(E)], axis=1
    )
    return out


# revision 6
# speedup vs baseline: 109.1737x; 3.4530x over previous
"""Experts-choose-contract MoE kernel for Trainium2 (8 NeuronCores).

Problem: x (B=4, T=4096, D=1024) f32; expert_indices (B, E=8, C=1024);
weight (E, O=512, D); bias (E, O).
out[b, e, c, :] = x[b, expert_indices[b, e, c], :] @ weight[e].T + bias[e]

Sharding: expert-parallel — core e handles expert e. x is replicated; each
core gathers its expert's (B*C = 4096) token rows from HBM with dma_gather,
transposes token tiles on the PE (contract dim must sit on partitions),
runs the grouped GEMM (4096x1024 @ 1024x512) accumulating in PSUM, adds
bias, and writes its (4096, 512) slice. The host stacks the 8 slices.

Modes (env BASSK_MODE): "f32" exact fp32 matmul (4 cyc/row),
"f32r" (default) single-pass fp32 matmul (1 cyc/row at N>=512),
"bf16" host-casts x/w to bf16 and gathers pre-transposed (2-byte dtype
supports transposing gather), skipping the PE transposes entirely.
"""

import os

import numpy as np

import concourse.bass as bass
import concourse.mybir as mybir
import concourse.tile as tile
from concourse import bacc
from concourse.bass_utils import run_bass_kernel_spmd
from concourse.masks import make_identity

B, T, D = 4, 4096, 1024
E, C, O = 8, 1024, 512
BT = B * T          # 16384 rows in flattened x
NTOK = B * C        # 4096 tokens gathered per expert/core
KT = D // 128       # 8 contraction tiles
CHUNK = 512         # tokens per dma_gather
NCHUNK = NTOK // CHUNK
IDX_COLS = NTOK // 16

MODE = os.environ.get("BASSK_MODE", "f32r")


def build_nc(mode=MODE, repeat=1):
    nc = bacc.Bacc("TRN2", target_bir_lowering=False, debug=False)
    f32 = mybir.dt.float32
    bf16 = mybir.dt.bfloat16
    i16 = mybir.dt.int16

    mm_dt = {
        "f32": f32,
        "f32r": mybir.dt.float32r,
        "bf16": bf16,
    }[mode]

    if mode == "bf16":
        x_dram = nc.dram_tensor("x", [BT, D], bf16, kind="ExternalInput")
    else:
        x_dram = nc.dram_tensor("x", [BT, D], f32, kind="ExternalInput")
    wt_dram = nc.dram_tensor("wt", [128, KT, O], mm_dt, kind="ExternalInput")
    idx_dram = nc.dram_tensor("idx", [128, IDX_COLS], i16, kind="ExternalInput")
    bias_dram = nc.dram_tensor("bias", [O], f32, kind="ExternalInput")
    out_dram = nc.dram_tensor("out", [NTOK, O], f32, kind="ExternalOutput")

    with tile.TileContext(nc) as tc:
        with (
            tc.tile_pool(name="singles", bufs=1) as singles,
            tc.tile_pool(name="gpool", bufs=2) as gpool,
            tc.tile_pool(name="tpool", bufs=3) as tpool,
            tc.tile_pool(name="opool", bufs=3) as opool,
            tc.tile_pool(name="psum_t", bufs=4, space="PSUM") as psum_t,
            tc.tile_pool(name="psum_mm", bufs=3, space="PSUM") as psum_mm,
        ):
            wt_sb = singles.tile([128, KT, O], wt_dram.dtype)
            nc.sync.dma_start(wt_sb, wt_dram.ap())
            bias_sb = singles.tile([128, O], f32)
            nc.sync.dma_start(
                bias_sb,
                bass.AP(tensor=bias_dram, offset=0, ap=[[0, 128], [1, O]]),
            )
            idx_sb = singles.tile([128, IDX_COLS], i16)
            nc.sync.dma_start(idx_sb, idx_dram.ap())
            if mode != "bf16":
                ident = singles.tile([128, 128], f32)
                make_identity(nc, ident)

            for c in range(NCHUNK * repeat):
                c = c % NCHUNK
                icols = CHUNK // 16
                idx_slice = idx_sb[:, c * icols : (c + 1) * icols]
                if mode == "bf16":
                    # transposing gather: g[p, k, t] = x[tok_t, k*128 + p]
                    g = gpool.tile([128, KT, CHUNK], bf16)
                    nc.gpsimd.dma_gather(
                        out_ap=g[:],
                        in_ap=x_dram.ap(),
                        idxs_ap=idx_slice,
                        num_idxs=CHUNK,
                        num_idxs_reg=CHUNK,
                        elem_size=D,
                        transpose=True,
                    )
                else:
                    # g[p, j, :] = token row (c*CHUNK + j*128 + p)
                    g = gpool.tile([128, CHUNK // 128, D], f32)
                    nc.gpsimd.dma_gather(
                        out_ap=g[:],
                        in_ap=x_dram.ap(),
                        idxs_ap=idx_slice,
                        num_idxs=CHUNK,
                        num_idxs_reg=CHUNK,
                        elem_size=D,
                    )

                if mode == "bf16":
                    # matmul directly from the transposed gather, 512-token N
                    # split into PSUM-bank-sized 512 outputs: out tile is
                    # [tok, O] so tokens must be the PSUM partition dim ->
                    # need lhsT = tokens. g[:, k, :] is [d128, tok512];
                    # use it as lhsT in 128-token column slices.
                    for j in range(CHUNK // 128):
                        pso = psum_mm.tile([128, O], f32)
                        for k in range(KT):
                            nc.tensor.matmul(
                                pso,
                                lhsT=g[:, k, j * 128 : (j + 1) * 128],
                                rhs=wt_sb[:, k, :],
                                start=(k == 0),
                                stop=(k == KT - 1),
                            )
                        ot = opool.tile([128, O], f32)
                        nc.vector.tensor_add(ot, pso, bias_sb)
                        t = c * (CHUNK // 128) + j
                        nc.sync.dma_start(
                            out_dram.ap()[t * 128 : (t + 1) * 128, :], ot
                        )
                else:
                    for j in range(CHUNK // 128):
                        tokT = tpool.tile([128, KT, 128], mm_dt)
                        for k in range(KT):
                            pst = psum_t.tile([128, 128], f32)
                            nc.tensor.transpose(
                                pst, g[:, j, k * 128 : (k + 1) * 128], ident
                            )
                            # alternate copy engines to split the PSUM->SBUF load
                            # (these also perform the f32 -> f32r rounding)
                            if k % 2 == 0:
                                nc.vector.tensor_copy(tokT[:, k, :], pst)
                            else:
                                nc.scalar.copy(tokT[:, k, :], pst)
                        pso = psum_mm.tile([128, O], f32)
                        for k in range(KT):
                            nc.tensor.matmul(
                                pso,
                                lhsT=tokT[:, k, :],
                                rhs=wt_sb[:, k, :],
                                start=(k == 0),
                                stop=(k == KT - 1),
                            )
                        ot = opool.tile([128, O], f32)
                        nc.vector.tensor_add(ot, pso, bias_sb)
                        t = c * (CHUNK // 128) + j
                        nc.sync.dma_start(
                            out_dram.ap()[t * 128 : (t + 1) * 128, :], ot
                        )

    nc.compile()
    return nc


def prepare_in_maps(x, expert_indices, weight, bias, mode=MODE):
    x = np.ascontiguousarray(np.asarray(x, dtype=np.float32).reshape(BT, D))
    idx = np.asarray(expert_indices).astype(np.int64)
    w = np.asarray(weight, dtype=np.float32)
    b = np.asarray(bias, dtype=np.float32)

    if mode == "bf16":
        import ml_dtypes

        x_dev = x.astype(ml_dtypes.bfloat16)
    else:
        x_dev = x

    in_maps = []
    for e in range(E):
        flat = (idx[:, e, :] + (np.arange(B) * T)[:, None]).reshape(-1)
        assert flat.min() >= 0 and flat.max() < BT
        flat = flat.astype(np.int16)
        # per-chunk wrap: index i of chunk c sits at [i % 16, c*icols + i//16],
        # replicated to all 8 Q7 core groups (128 partitions).
        wrapped = flat.reshape(NCHUNK, CHUNK // 16, 16).transpose(2, 0, 1)
        idx_arr = np.tile(wrapped.reshape(16, IDX_COLS), (8, 1)).copy()
        wt = np.ascontiguousarray(
            w[e].T.reshape(KT, 128, O).transpose(1, 0, 2)
        )  # [128, KT, O]; wt[p, k, o] = w[e, o, k*128+p]
        if mode == "bf16":
            import ml_dtypes

            wt = wt.astype(ml_dtypes.bfloat16)
        in_maps.append(
            {"x": x_dev, "idx": idx_arr, "wt": wt, "bias": b[e].copy()}
        )
    return in_maps


_NC_CACHE = {}


def _get_nc(mode=MODE):
    if mode not in _NC_CACHE:
        _NC_CACHE[mode] = build_nc(mode)
    return _NC_CACHE[mode]


def kernel(x, expert_indices, weight, bias, _collect=None):
    nc = _get_nc()
    in_maps = prepare_in_maps(x, expert_indices, weight, bias)
    kwargs = {}
    if _collect is not None:
        kwargs = _collect.pop("kwargs", {})
    res = run_bass_kernel_spmd(nc, in_maps, core_ids=list(range(E)), **kwargs)
    if _collect is not None:
        _collect["res"] = res
    out = np.stack(
        [res.results[e]["out"].reshape(B, C, O) for e in range(E)], axis=1
    )
    return out
